# revision 2
# baseline (speedup 1.0000x reference)
# CARAFE Trainium2 kernel, v2 — bf16 compute, channel-major gather windows,
# PE band-matmul taps + DVE STT taps, direct psum->HBM output.
#
# Per core (one batch item):
#   conv1 (1x1+BN+SiLU) on PE/ACT/DVE -> t_pad bf16 [64, 66*66]
#   conv2 (3x3+BN) in pixel-major orientation: psum [128px, 100] per 2-row tile
#   softmax over 25 taps per (subpixel-class, pixel) via DVE reduce/recip
#   reassembly: 25 taps x 4 classes per tile:
#     PE-route: band matrix [128, 512] (mask values on a selector band, built by
#       Pool affine_select or DVE tensor_tensor vs static SEL) x window -> psum
#     DVE-route: per-class scalar_tensor_tensor FMA chains (bf16 4x mode)
#       then transposed into psum via static permutation matmuls
#   psum [c_chunk, 512] holds 4 output rows (di,dh,w,dj)-ordered -> single DMA
import sys

import numpy as np

for _p in ("/opt/trn_rl_repo",):
    if _p not in sys.path:
        sys.path.insert(0, _p)

B, C, Cm, E = 8, 192, 64, 100
H = W = 64
K, S = 5, 2
EPS = 1e-3
NT = 32
XCOLS = 64 * 66 + 4   # 2 front zeros + 64*64*... wait: per-row 64 cols; 64 rows = 4096 + 2 front + 2 tail
XLEN = 4100

# tap routing: most (i, j) on PE via band matmuls; band source 'pool'
# (affine_select) or 'dve' (tensor_tensor vs static SEL). A few taps run as
# direct STT FMA chains on Pool or DVE (POOL_TAPS / remainder).
PE_TAPS = {}
for _i in range(5):
    for _j in range(5):
        if (_i, _j) in ((0, 0), (0, 4), (4, 0), (4, 4), (2, 0)):
            continue
        PE_TAPS[(_i, _j)] = 'pool' if (_i, _j) in ((0, 1), (0, 2), (0, 3), (1, 0), (1, 4)) else 'dve'
POOL_TAPS = []
DVE_TAPS = [(0, 4), (4, 0), (2, 0), (0, 0), (4, 4)]
# order so that first and last PE tap in every tile are always-valid (i=2 row)
_prog_cache = {}


def _build_program(num_devices=8):
    import concourse.mybir as mybir
    import concourse.tile as tile
    from concourse import bacc
    from contextlib import ExitStack

    fp32 = mybir.dt.float32
    nc = bacc.Bacc("TRN2", target_bir_lowering=False, num_devices=num_devices)

    x_d = nc.dram_tensor("x", [C, H * W], fp32, kind="ExternalInput").ap()
    cw_d = nc.dram_tensor("cw", [C, Cm], fp32, kind="ExternalInput").ap()
    cb_d = nc.dram_tensor("cb", [Cm, 1], fp32, kind="ExternalInput").ap()
    ew_d = nc.dram_tensor("ew", [Cm, 9 * E], fp32, kind="ExternalInput").ap()
    eb_d = nc.dram_tensor("eb", [1, E], fp32, kind="ExternalInput").ap()
    edge_d = nc.dram_tensor("edge", [128, K * K], fp32, kind="ExternalInput").ap()
    ident_d = nc.dram_tensor("idm", [128, 128], fp32, kind="ExternalInput").ap()
    sel_d = nc.dram_tensor("sel", [128, 512], fp32, kind="ExternalInput").ap()
    perm_d = nc.dram_tensor("perm", [128, 2 * 256], fp32, kind="ExternalInput").ap()
    out_d = nc.dram_tensor("out", [C, 2 * H, 2 * W], fp32, kind="ExternalOutput").ap()

    es = ExitStack()
    with tile.TileContext(nc) as tc:
        with es:
            _body(es, tc, nc, mybir, x_d, cw_d, cb_d, ew_d, eb_d, edge_d,
                  ident_d, sel_d, perm_d, out_d)
    nc.compile()
    return nc


def _body(es, tc, nc, mybir, x_d, cw_d, cb_d, ew_d, eb_d, edge_d,
          ident_d, sel_d, perm_d, out_d):
    fp32 = mybir.dt.float32
    bf16 = mybir.dt.bfloat16
    AL = mybir.AluOpType
    AF = mybir.ActivationFunctionType

    consts = es.enter_context(tc.tile_pool(name="consts", bufs=1))
    big = es.enter_context(tc.tile_pool(name="big", bufs=1))

    # fp32 staging for consts
    stg = consts.tile([128, 1536], fp32, tag="cstg")
    cw0 = consts.tile([128, Cm], bf16, tag="cw0")
    cw1 = consts.tile([64, Cm], bf16, tag="cw1")
    cb = consts.tile([Cm, 1], fp32, tag="cb")
    ew = consts.tile([Cm, 9 * E], bf16, tag="ew")
    ewf = consts.tile([Cm, 9 * E], fp32, tag="ewf")
    ebrow = consts.tile([1, E], bf16, tag="ebrow")
    ones1 = consts.tile([1, 128], bf16, tag="ones1")
    edge = consts.tile([128, K * K], fp32, tag="edge")
    ident = consts.tile([128, 128], bf16, tag="ident")
    sel = consts.tile([128, 512], bf16, tag="sel")
    perm = consts.tile([128, 2, 256], bf16, tag="perm")
    zwin = consts.tile([128, 192], bf16, tag="zwin")

    xsb = big.tile([128, 2 * 2048], fp32, tag="xsb")       # staging chunk-rotated
    xb = big.tile([128, 2 * XLEN], bf16, tag="xb")         # [0:4100]=c0..127, [4100:8200]=c128..191 (rows 64+: zero)
    t_pad = big.tile([Cm, 66 * 66], bf16, tag="tpad")
    maskT = big.tile([128, NT, E], fp32, tag="maskT")
    maskTb = big.tile([128, NT, E], bf16, tag="maskTb")

    # ---- const DMAs + conversions ----
    nc.sync.dma_start(out=stg[:, 0:Cm], in_=cw_d[0:128, :])
    nc.scalar.copy(out=cw0[:], in_=stg[:, 0:Cm])
    nc.sync.dma_start(out=stg[0:64, 64:128], in_=cw_d[128:192, :])
    nc.scalar.copy(out=cw1[:], in_=stg[0:64, 64:128])
    nc.sync.dma_start(out=cb[:], in_=cb_d)
    nc.sync.dma_start(out=ewf[:], in_=ew_d)
    nc.scalar.copy(out=ew[:], in_=ewf[:])
    nc.sync.dma_start(out=stg[0:1, 128:228], in_=eb_d)
    nc.scalar.copy(out=ebrow[:], in_=stg[0:1, 128:228])
    nc.sync.dma_start(out=edge[:], in_=edge_d)
    nc.sync.dma_start(out=stg[:, 256:384], in_=ident_d)
    nc.scalar.copy(out=ident[:], in_=stg[:, 256:384])
    nc.sync.dma_start(out=stg[:, 384:896], in_=sel_d)
    nc.scalar.copy(out=sel[:], in_=stg[:, 384:896])
    nc.sync.dma_start(out=stg[:, 896:1408], in_=perm_d)
    nc.scalar.copy(out=perm[:].rearrange("p a b -> p (a b)"), in_=stg[:, 896:1408])
    nc.gpsimd.memset(ones1[:], 1.0)
    nc.gpsimd.memset(zwin[:], 0.0)
    nc.gpsimd.memset(xb[:], 0.0)
    nc.gpsimd.memset(t_pad[:], 0.0)

    # ---- x load + bf16 convert: 8 chunks of 512 cols ----
    for k in range(8):
        c0 = k * 512
        xs = xsb[:, (k % 2) * 2048:(k % 2) * 2048 + 1024]
        nc.sync.dma_start(out=xs[:, 0:512], in_=x_d[0:128, c0:c0 + 512])
        nc.sync.dma_start(out=xs[0:64, 512:1024], in_=x_d[128:192, c0:c0 + 512])
        nc.scalar.copy(out=xb[:, 2 + c0:2 + c0 + 512], in_=xs[:, 0:512])
        nc.scalar.copy(out=xb[0:64, XLEN + 2 + c0:XLEN + 2 + c0 + 512], in_=xs[0:64, 512:1024])

    # ---- psum pools (keep all open; 8 banks total) ----
    c2ps = es.enter_context(tc.tile_pool(name="c2ps", bufs=2, space="PSUM"))
    wps = es.enter_context(tc.tile_pool(name="wps", bufs=2, space="PSUM"))
    ops0 = es.enter_context(tc.tile_pool(name="ops0", bufs=2, space="PSUM"))
    ops1 = es.enter_context(tc.tile_pool(name="ops1", bufs=2, space="PSUM"))
    sgp = es.enter_context(tc.tile_pool(name="sgp", bufs=4))
    winp = es.enter_context(tc.tile_pool(name="winp", bufs=44))
    ostgp = es.enter_context(tc.tile_pool(name="ostgp", bufs=3))
    bandp = es.enter_context(tc.tile_pool(name="bandp", bufs=42))
    accp = es.enter_context(tc.tile_pool(name="accp", bufs=10))

    # ---- conv1 + SiLU -> t_pad interior ----
    def conv1(ntk):
        n0 = ntk * 512
        ps = c2ps.tile([128, 512], fp32, name="c1ps", tag="c2")
        nc.tensor.matmul(ps[0:Cm, :], cw0[:], xb[:, 2 + n0:2 + n0 + 512], start=True, stop=False)
        nc.tensor.matmul(ps[0:Cm, :], cw1[:], xb[0:64, XLEN + 2 + n0:XLEN + 2 + n0 + 512], start=False, stop=True)
        sg = sgp.tile([Cm, 512], fp32, tag="sg")
        nc.scalar.activation(out=sg[:], in_=ps[0:Cm, :], func=AF.Sigmoid, bias=cb[:], scale=1.0)
        v = t_pad[:].rearrange("c (r z) -> c r z", z=66)[:, ntk * 8 + 1: ntk * 8 + 9, 1:65]
        nc.vector.scalar_tensor_tensor(
            v, ps[0:Cm, :].rearrange("c (r z) -> c r z", z=64), cb[:],
            sg[:].rearrange("c (r z) -> c r z", z=64), AL.add, AL.mult)

    # ---- conv2 + softmax mask, pixel-major ----
    def conv2(ti):
        r0 = ti * 2
        ps = c2ps.tile([128, 512], fp32, name="c2ps", tag="c2")
        tv = t_pad[:].rearrange("c (r z) -> c r z", z=66)
        for tap in range(9):
            dy, dx = tap // 3, tap % 3
            for dh in range(2):
                lhs = tv[:, r0 + dy + dh, dx:dx + 64]
                nc.tensor.matmul(ps[dh * 64:dh * 64 + 64, 0:E], lhs,
                                 ew[:, tap * E:(tap + 1) * E],
                                 start=(tap == 0), stop=False,
                                 skip_group_check=(tap > 0 or dh > 0))
        for dh in range(2):
            nc.tensor.matmul(ps[dh * 64:dh * 64 + 64, 0:E], ones1[:, 0:64], ebrow[:],
                             start=False, stop=True, skip_group_check=(dh > 0))
        e2 = sgp.tile([128, E], fp32, tag="e2")
        rsum = sgp.tile([128, 4], fp32, tag="rsum")
        norm = sgp.tile([128, K * K, 4], fp32, tag="norm")
        nc.scalar.activation(out=e2[:], in_=ps[:, 0:E], func=AF.Exp, bias=0.0, scale=1.0)
        ev = e2[:].rearrange("p (ij cl) -> p cl ij", cl=4)
        nc.vector.tensor_reduce(out=rsum[:], in_=ev, axis=mybir.AxisListType.X, op=AL.add)
        nc.vector.reciprocal(rsum[:], rsum[:])
        # norm[p, ij, cl] = edge[p, ij] * rinv[p, cl]
        e_b = edge[:].unsqueeze(2).broadcast_to([128, K * K, 4])
        r_b = rsum[:].unsqueeze(1).broadcast_to([128, K * K, 4])
        nc.vector.tensor_tensor(norm[:], e_b, r_b, AL.mult)
        nv = norm[:].rearrange("p ij cl -> p (ij cl)")
        nc.gpsimd.tensor_tensor(maskT[:, ti, :], e2[:], nv, AL.mult)
        nc.scalar.copy(out=maskTb[:, ti, :], in_=maskT[:, ti, :])

    # ---- windows ----
    win_cache = {}

    def evict(dst, src):
        nc.scalar.copy(out=dst, in_=src)

    def get_window(r, dj):
        if r <= -2 or r >= 64:
            return zwin
        key = (r, dj)
        if key in win_cache:
            return win_cache[key]
        w_sb = winp.tile([128, 192], bf16, tag="win")
        if r == -1:
            off = 2 + dj
            ps = wps.tile([128, 192], fp32, name="wps", tag="w")
            nc.tensor.matmul(ps[64:128, 0:128], xb[:, off:off + 64], ident[:], start=True, stop=False)
            nc.tensor.matmul(ps[64:128, 128:192], xb[0:64, XLEN + off:XLEN + off + 64],
                             ident[0:64, 0:64], start=False, stop=True)
            nc.gpsimd.memset(w_sb[0:64, :], 0.0)
            evict(w_sb[64:128, :], ps[64:128, 0:192])
        elif r == 63:
            off = 2 + 64 * 63 + dj
            ps = wps.tile([128, 192], fp32, name="wps", tag="w")
            nc.tensor.matmul(ps[0:64, 0:128], xb[:, off:off + 64], ident[:], start=True, stop=False)
            nc.tensor.matmul(ps[0:64, 128:192], xb[0:64, XLEN + off:XLEN + off + 64],
                             ident[0:64, 0:64], start=False, stop=True)
            nc.gpsimd.memset(w_sb[64:128, :], 0.0)
            evict(w_sb[0:64, :], ps[0:64, 0:192])
        else:
            off = 2 + 64 * r + dj
            ps = wps.tile([128, 192], fp32, name="wps", tag="w")
            nc.tensor.matmul(ps[:, 0:128], xb[:, off:off + 128], ident[:], start=True, stop=False)
            nc.tensor.matmul(ps[:, 128:192], xb[0:64, XLEN + off:XLEN + off + 128],
                             ident[0:64, 0:64], start=False, stop=True)
            evict(w_sb[:], ps[:, 0:192])
        win_cache[key] = w_sb
        return w_sb

    # ---- window & band prefetch ----
    def valid_taps(ti):
        h0 = ti * 2
        return {t: (h0 + t[0] - 2) for t in PE_TAPS if -2 < h0 + t[0] - 2 < 64 or t[0] == 2}

    def wins_for(ti):
        h0 = ti * 2
        for kk in list(win_cache.keys()):
            if kk[0] < h0 - 4:
                del win_cache[kk]
        for i in range(K):
            r = h0 + i - 2
            for j in range(K):
                get_window(r, j - 2)

    band_store = {}

    def bands(ti):
        for (i, j) in PE_TAPS:
            r = ti * 2 + i - 2
            if r <= -2 or r >= 64:
                continue
            ij = i * K + j
            band = bandp.tile([128, 512], bf16, tag="band")
            # col index f = di*256 + q*2 + t with q = dh*64 + w = band row; the
            # (dh, w) pair merges into one uniform-stride dim (3D ISA limit).
            ov = band[:].rearrange("p (di q t) -> p di q t", di=2, q=128, t=2)
            iv = maskTb[:, ti, ij * 4:ij * 4 + 4].rearrange("p (di t) -> p di t", di=2)
            iv = iv.unsqueeze(2).broadcast_to([128, 2, 128, 2])
            if PE_TAPS[(i, j)] == 'pool':
                for di in range(2):
                    ovd = band[:, di * 256:di * 256 + 256].rearrange(
                        "p (q t) -> p q t", q=128)
                    ivd = maskTb[:, ti, ij * 4 + di * 2:ij * 4 + di * 2 + 2]
                    ivd = ivd.unsqueeze(1).broadcast_to([128, 128, 2])
                    nc.gpsimd.affine_select(out=ovd, in_=ivd, compare_op=AL.is_equal,
                                            fill=0.0, base=0, channel_multiplier=1,
                                            pattern=[[-1, 128], [0, 2]])
            else:
                sv = sel[:].rearrange("p (di q t) -> p di q t", di=2, q=128, t=2)
                nc.vector.tensor_tensor(ov, iv, sv, AL.mult)
            band_store[(ti % 2, (i, j))] = band

    # ---- per-tile reassembly ----
    def taps(ti):
        h0 = ti * 2
        po0 = ops0.tile([128, 512], fp32, name="po0", tag="o0")
        po1 = ops1.tile([64, 512], fp32, name="po1", tag="o1")
        wins = {}
        for i in range(K):
            r = h0 + i - 2
            for j in range(K):
                wins[(i, j)] = get_window(r, j - 2)

        pe_list = [t for t in PE_TAPS if not (wins[t] is zwin)]
        # order: start with (2,1), end with (2,2) (always valid, full width)
        pe_list.sort(key=lambda t: (t != (2, 1), t == (2, 2)))
        assert pe_list[-1] == (2, 2) and pe_list[0] == (2, 1)
        dve_list = [t for t in DVE_TAPS if not (wins[t] is zwin)]

        # PE taps: first (start, full width)
        for k, (i, j) in enumerate(pe_list[:-1]):
            band = band_store.pop((ti % 2, (i, j)))
            w_sb = wins[(i, j)]
            st = (k == 0)
            nc.tensor.matmul(po0[:], w_sb[:, 0:128], band[:], start=st, stop=False,
                             skip_group_check=not st)
            nc.tensor.matmul(po1[:], w_sb[:, 128:192], band[:], start=st, stop=False,
                             skip_group_check=not st)

        # direct taps: DVE STT chains per class, transposed into psum via perms
        for cl in range(4):
            acc = accp.tile([128, 192], bf16, tag="acc")
            fd = True
            for (i, j) in dve_list:
                ij = i * K + j
                col = maskT[:, ti, ij * 4 + cl:ij * 4 + cl + 1]
                src_ = zwin if fd else acc
                nc.vector.scalar_tensor_tensor(acc[:], wins[(i, j)][:], col, src_[:], AL.mult, AL.add)
                fd = False
            if fd:
                continue
            di, dj = cl // 2, cl % 2
            nc.tensor.matmul(po0[:, di * 256:di * 256 + 256], acc[:, 0:128], perm[:, dj, :],
                             start=False, stop=False, skip_group_check=True)
            nc.tensor.matmul(po1[:, di * 256:di * 256 + 256], acc[:, 128:192], perm[:, dj, :],
                             start=False, stop=False, skip_group_check=True)

        # final PE tap (2,2): stop, full width
        i, j = pe_list[-1]
        band = band_store.pop((ti % 2, (i, j)))
        w_sb = wins[(i, j)]
        nc.tensor.matmul(po0[:], w_sb[:, 0:128], band[:], start=False, stop=True)
        nc.tensor.matmul(po1[:], w_sb[:, 128:192], band[:], start=False, stop=True)

        # evict psum -> SBUF staging (DMA cannot read PSUM), then DMA out.
        os0 = ostgp.tile([128, 512], fp32, tag="os0")
        os1 = ostgp.tile([64, 512], fp32, tag="os1")
        nc.scalar.copy(out=os0[:], in_=po0[:])
        nc.scalar.copy(out=os1[:], in_=po1[:])
        for di in range(2):
            ov0 = out_d[0:128, 2 * h0 + di:2 * h0 + di + 3:2, :]
            nc.sync.dma_start(out=ov0, in_=os0[:, di * 256:di * 256 + 256]
                              .rearrange("c (dh q) -> c dh q", dh=2))
            ov1 = out_d[128:192, 2 * h0 + di:2 * h0 + di + 3:2, :]
            nc.sync.dma_start(out=ov1, in_=os1[:, di * 256:di * 256 + 256]
                              .rearrange("c (dh q) -> c dh q", dh=2))

    for ntk in range(8):
        conv1(ntk)
    conv2(0)
    conv2(1)
    bands(0)
    wins_for(0)
    for ti in range(NT):
        if ti + 2 < NT:
            conv2(ti + 2)
        if ti + 1 < NT:
            bands(ti + 1)
            wins_for(ti + 1)
        taps(ti)
    es.pop_all().close()


def _host_prep(inputs):
    def fold(w, g, b, m, v):
        s = g / np.sqrt(v + EPS)
        return (w * s[:, None, None, None]).astype(np.float32), (b - m * s).astype(np.float32)

    comp_w_eff, comp_b_eff = fold(inputs["comp_w"], inputs["comp_g"], inputs["comp_b"],
                                  inputs["comp_m"], inputs["comp_v"])
    enc_w_eff, enc_b_eff = fold(inputs["enc_w"], inputs["enc_g"], inputs["enc_b"],
                                inputs["enc_m"], inputs["enc_v"])
    cw = np.ascontiguousarray(comp_w_eff[:, :, 0, 0].T)          # [192, 64]
    cb = comp_b_eff.reshape(Cm, 1)
    ew = np.concatenate([enc_w_eff[:, :, dy, dx].T
                         for dy in range(3) for dx in range(3)], axis=1)  # [64, 900]
    ew = np.ascontiguousarray(ew)
    eb = enc_b_eff.reshape(1, E)
    wv = np.arange(128) % 64
    edge = np.zeros((128, K * K), np.float32)
    for j in range(K):
        ok = (wv + j - 2 >= 0) & (wv + j - 2 < W)
        for i in range(K):
            edge[:, i * K + j] = ok
    idm = np.eye(128, dtype=np.float32)
    sel = np.zeros((128, 512), np.float32)
    for f in range(512):
        dh, w = (f // 128) % 2, (f % 128) // 2
        sel[64 * dh + w, f] = 1.0
    perm = np.zeros((128, 2, 256), np.float32)
    for p in range(128):
        dh, w = p // 64, p % 64
        for dj in range(2):
            perm[p, dj, dh * 128 + 2 * w + dj] = 1.0
    perm = perm.reshape(128, 512)
    return dict(cw=cw, cb=cb, ew=ew, eb=eb, edge=edge, idm=idm, sel=sel, perm=perm)


def kernel(**inputs):
    from concourse.bass_utils import run_bass_kernel_spmd

    inputs = {k: np.asarray(v, dtype=np.float32) for k, v in inputs.items()}
    w = _host_prep(inputs)
    if "nc" not in _prog_cache:
        _prog_cache["nc"] = _build_program()
    nc = _prog_cache["nc"]
    x = inputs["x"]
    in_maps = [dict(x=np.ascontiguousarray(x[b].reshape(C, H * W)), **w) for b in range(B)]
    res = run_bass_kernel_spmd(nc, in_maps, list(range(B)))
    out = np.stack([res.results[b]["out"] for b in range(B)])
    return out


# revision 6
# speedup vs baseline: 1.0514x; 1.0514x over previous
# CARAFE Trainium2 kernel, v2 — bf16 compute, channel-major gather windows,
# PE band-matmul taps + DVE STT taps, direct psum->HBM output.
#
# Per core (one batch item):
#   conv1 (1x1+BN+SiLU) on PE/ACT/DVE -> t_pad bf16 [64, 66*66]
#   conv2 (3x3+BN) in pixel-major orientation: psum [128px, 100] per 2-row tile
#   softmax over 25 taps per (subpixel-class, pixel) via DVE reduce/recip
#   reassembly: 25 taps x 4 classes per tile:
#     PE-route: band matrix [128, 512] (mask values on a selector band, built by
#       Pool affine_select or DVE tensor_tensor vs static SEL) x window -> psum
#     DVE-route: per-class scalar_tensor_tensor FMA chains (bf16 4x mode)
#       then transposed into psum via static permutation matmuls
#   psum [c_chunk, 512] holds 4 output rows (di,dh,w,dj)-ordered -> single DMA
import sys

import numpy as np

for _p in ("/opt/trn_rl_repo",):
    if _p not in sys.path:
        sys.path.insert(0, _p)

B, C, Cm, E = 8, 192, 64, 100
H = W = 64
K, S = 5, 2
EPS = 1e-3
NT = 32
XCOLS = 64 * 66 + 4   # 2 front zeros + 64*64*... wait: per-row 64 cols; 64 rows = 4096 + 2 front + 2 tail
XLEN = 4100

# tap routing: most (i, j) on PE via band matmuls; band source 'pool'
# (affine_select) or 'dve' (tensor_tensor vs static SEL). A few taps run as
# direct STT FMA chains on Pool or DVE (POOL_TAPS / remainder).
PE_TAPS = {}
for _i in range(5):
    for _j in range(5):
        if (_i, _j) in ((0, 0), (0, 4), (4, 0), (4, 4), (2, 0), (3, 1)):
            continue
        PE_TAPS[(_i, _j)] = 'pool' if (_i, _j) in ((0, 1), (0, 2), (0, 3), (1, 0), (1, 4), (4, 1), (4, 3)) else 'dve'
POOL_TAPS = []
DVE_TAPS = [(0, 4), (4, 0), (2, 0), (0, 0), (4, 4), (3, 1)]
# order so that first and last PE tap in every tile are always-valid (i=2 row)
_prog_cache = {}


def _build_program(num_devices=8):
    import concourse.mybir as mybir
    import concourse.tile as tile
    from concourse import bacc
    from contextlib import ExitStack

    fp32 = mybir.dt.float32
    nc = bacc.Bacc("TRN2", target_bir_lowering=False, num_devices=num_devices)

    x_d = nc.dram_tensor("x", [C, H * W], fp32, kind="ExternalInput").ap()
    cw_d = nc.dram_tensor("cw", [C, Cm], fp32, kind="ExternalInput").ap()
    cb_d = nc.dram_tensor("cb", [Cm, 1], fp32, kind="ExternalInput").ap()
    ew_d = nc.dram_tensor("ew", [Cm, 9 * E], fp32, kind="ExternalInput").ap()
    eb_d = nc.dram_tensor("eb", [1, E], fp32, kind="ExternalInput").ap()
    edge_d = nc.dram_tensor("edge", [128, K * K], fp32, kind="ExternalInput").ap()
    ident_d = nc.dram_tensor("idm", [128, 128], fp32, kind="ExternalInput").ap()
    sel_d = nc.dram_tensor("sel", [128, 512], fp32, kind="ExternalInput").ap()
    perm_d = nc.dram_tensor("perm", [128, 2 * 256], fp32, kind="ExternalInput").ap()
    out_d = nc.dram_tensor("out", [C, 2 * H, 2 * W], fp32, kind="ExternalOutput").ap()

    es = ExitStack()
    with tile.TileContext(nc) as tc:
        with es:
            _body(es, tc, nc, mybir, x_d, cw_d, cb_d, ew_d, eb_d, edge_d,
                  ident_d, sel_d, perm_d, out_d)
    nc.compile()
    return nc


def _body(es, tc, nc, mybir, x_d, cw_d, cb_d, ew_d, eb_d, edge_d,
          ident_d, sel_d, perm_d, out_d):
    fp32 = mybir.dt.float32
    bf16 = mybir.dt.bfloat16
    AL = mybir.AluOpType
    AF = mybir.ActivationFunctionType

    consts = es.enter_context(tc.tile_pool(name="consts", bufs=1))
    big = es.enter_context(tc.tile_pool(name="big", bufs=1))

    # fp32 staging for consts
    stg = consts.tile([128, 1536], fp32, tag="cstg")
    cw0 = consts.tile([128, Cm], bf16, tag="cw0")
    cw1 = consts.tile([64, Cm], bf16, tag="cw1")
    cb = consts.tile([Cm, 1], fp32, tag="cb")
    ew = consts.tile([Cm, 9 * E], bf16, tag="ew")
    ewf = consts.tile([Cm, 9 * E], fp32, tag="ewf")
    ebrow = consts.tile([1, E], bf16, tag="ebrow")
    ones1 = consts.tile([1, 128], bf16, tag="ones1")
    edge = consts.tile([128, K * K], fp32, tag="edge")
    ident = consts.tile([128, 128], bf16, tag="ident")
    sel = consts.tile([128, 512], bf16, tag="sel")
    perm = consts.tile([128, 2, 256], bf16, tag="perm")
    zwin = consts.tile([128, 192], bf16, tag="zwin")

    xsb = big.tile([128, 2 * 2048], fp32, tag="xsb")       # staging chunk-rotated
    xb = big.tile([128, 2 * XLEN], bf16, tag="xb")         # [0:4100]=c0..127, [4100:8200]=c128..191 (rows 64+: zero)
    t_pad = big.tile([Cm, 66 * 66], bf16, tag="tpad")
    maskT = big.tile([128, NT, E], fp32, tag="maskT")
    maskTb = big.tile([128, NT, E], bf16, tag="maskTb")

    # ---- const DMAs + conversions ----
    nc.sync.dma_start(out=stg[:, 0:Cm], in_=cw_d[0:128, :])
    nc.scalar.copy(out=cw0[:], in_=stg[:, 0:Cm])
    nc.sync.dma_start(out=stg[0:64, 64:128], in_=cw_d[128:192, :])
    nc.scalar.copy(out=cw1[:], in_=stg[0:64, 64:128])
    nc.sync.dma_start(out=cb[:], in_=cb_d)
    nc.sync.dma_start(out=ewf[:], in_=ew_d)
    nc.scalar.copy(out=ew[:], in_=ewf[:])
    nc.sync.dma_start(out=stg[0:1, 128:228], in_=eb_d)
    nc.scalar.copy(out=ebrow[:], in_=stg[0:1, 128:228])
    nc.sync.dma_start(out=edge[:], in_=edge_d)
    nc.sync.dma_start(out=stg[:, 256:384], in_=ident_d)
    nc.scalar.copy(out=ident[:], in_=stg[:, 256:384])
    nc.sync.dma_start(out=stg[:, 384:896], in_=sel_d)
    nc.scalar.copy(out=sel[:], in_=stg[:, 384:896])
    nc.sync.dma_start(out=stg[:, 896:1408], in_=perm_d)
    nc.scalar.copy(out=perm[:].rearrange("p a b -> p (a b)"), in_=stg[:, 896:1408])
    nc.gpsimd.memset(ones1[:], 1.0)
    nc.gpsimd.memset(zwin[:], 0.0)
    nc.gpsimd.memset(xb[:], 0.0)
    nc.gpsimd.memset(t_pad[:], 0.0)

    # ---- x load + bf16 convert: 8 chunks of 512 cols ----
    for k in range(8):
        c0 = k * 512
        xs = xsb[:, (k % 2) * 2048:(k % 2) * 2048 + 1024]
        nc.sync.dma_start(out=xs[:, 0:512], in_=x_d[0:128, c0:c0 + 512])
        nc.sync.dma_start(out=xs[0:64, 512:1024], in_=x_d[128:192, c0:c0 + 512])
        nc.scalar.copy(out=xb[:, 2 + c0:2 + c0 + 512], in_=xs[:, 0:512])
        nc.scalar.copy(out=xb[0:64, XLEN + 2 + c0:XLEN + 2 + c0 + 512], in_=xs[0:64, 512:1024])

    # ---- psum pools (keep all open; 8 banks total) ----
    c2ps = es.enter_context(tc.tile_pool(name="c2ps", bufs=1, space="PSUM"))
    wps = es.enter_context(tc.tile_pool(name="wps", bufs=3, space="PSUM"))
    ops0 = es.enter_context(tc.tile_pool(name="ops0", bufs=2, space="PSUM"))
    ops1 = es.enter_context(tc.tile_pool(name="ops1", bufs=2, space="PSUM"))
    sgp = es.enter_context(tc.tile_pool(name="sgp", bufs=4))
    winp = es.enter_context(tc.tile_pool(name="winp", bufs=26))
    ostgp = es.enter_context(tc.tile_pool(name="ostgp", bufs=3))
    bandp = es.enter_context(tc.tile_pool(name="bandp", bufs=42))
    accp = es.enter_context(tc.tile_pool(name="accp", bufs=10))

    # ---- conv1 + SiLU -> t_pad interior ----
    def conv1(ntk):
        n0 = ntk * 512
        ps = c2ps.tile([128, 512], fp32, name="c1ps", tag="c2")
        nc.tensor.matmul(ps[0:Cm, :], cw0[:], xb[:, 2 + n0:2 + n0 + 512], start=True, stop=False)
        nc.tensor.matmul(ps[0:Cm, :], cw1[:], xb[0:64, XLEN + 2 + n0:XLEN + 2 + n0 + 512], start=False, stop=True)
        sg = sgp.tile([Cm, 512], fp32, tag="sg")
        nc.scalar.activation(out=sg[:], in_=ps[0:Cm, :], func=AF.Sigmoid, bias=cb[:], scale=1.0)
        v = t_pad[:].rearrange("c (r z) -> c r z", z=66)[:, ntk * 8 + 1: ntk * 8 + 9, 1:65]
        nc.vector.scalar_tensor_tensor(
            v, ps[0:Cm, :].rearrange("c (r z) -> c r z", z=64), cb[:],
            sg[:].rearrange("c (r z) -> c r z", z=64), AL.add, AL.mult)

    # ---- conv2 + softmax mask, pixel-major ----
    def conv2(ti):
        r0 = ti * 2
        ps = c2ps.tile([128, 512], fp32, name="c2ps", tag="c2")
        tv = t_pad[:].rearrange("c (r z) -> c r z", z=66)
        for tap in range(9):
            dy, dx = tap // 3, tap % 3
            for dh in range(2):
                lhs = tv[:, r0 + dy + dh, dx:dx + 64]
                nc.tensor.matmul(ps[dh * 64:dh * 64 + 64, 0:E], lhs,
                                 ew[:, tap * E:(tap + 1) * E],
                                 start=(tap == 0), stop=False,
                                 skip_group_check=(tap > 0 or dh > 0))
        for dh in range(2):
            nc.tensor.matmul(ps[dh * 64:dh * 64 + 64, 0:E], ones1[:, 0:64], ebrow[:],
                             start=False, stop=True, skip_group_check=(dh > 0))
        e2 = sgp.tile([128, E], fp32, tag="e2")
        rsum = sgp.tile([128, 4], fp32, tag="rsum")
        norm = sgp.tile([128, K * K, 4], fp32, tag="norm")
        nc.scalar.activation(out=e2[:], in_=ps[:, 0:E], func=AF.Exp, bias=0.0, scale=1.0)
        ev = e2[:].rearrange("p (ij cl) -> p cl ij", cl=4)
        nc.vector.tensor_reduce(out=rsum[:], in_=ev, axis=mybir.AxisListType.X, op=AL.add)
        nc.vector.reciprocal(rsum[:], rsum[:])
        # norm[p, ij, cl] = edge[p, ij] * rinv[p, cl]
        e_b = edge[:].unsqueeze(2).broadcast_to([128, K * K, 4])
        r_b = rsum[:].unsqueeze(1).broadcast_to([128, K * K, 4])
        nc.gpsimd.tensor_tensor(norm[:], e_b, r_b, AL.mult)
        nv = norm[:].rearrange("p ij cl -> p (ij cl)")
        nc.gpsimd.tensor_tensor(maskT[:, ti, :], e2[:], nv, AL.mult)
        nc.scalar.copy(out=maskTb[:, ti, :], in_=maskT[:, ti, :])

    # ---- windows ----
    win_cache = {}  # (r, dj) -> (sbuf tile, col offset)

    def _build_edge(r, dj):
        # partial windows at image top/bottom: one valid row, other zeroed
        w_sb = winp.tile([128, 384], bf16, tag="win")
        ps = wps.tile([128, 512], fp32, name="wps", tag="w")
        if r == -1:
            off = 2 + dj
            nc.tensor.matmul(ps[64:128, 0:128], xb[:, off:off + 64], ident[:], start=True, stop=False)
            nc.tensor.matmul(ps[64:128, 128:192], xb[0:64, XLEN + off:XLEN + off + 64],
                             ident[0:64, 0:64], start=False, stop=True)
            nc.gpsimd.memset(w_sb[0:64, 0:192], 0.0)
            nc.scalar.copy(out=w_sb[64:128, 0:192], in_=ps[64:128, 0:192])
        else:  # r == 63
            off = 2 + 64 * 63 + dj
            nc.tensor.matmul(ps[0:64, 0:128], xb[:, off:off + 64], ident[:], start=True, stop=False)
            nc.tensor.matmul(ps[0:64, 128:192], xb[0:64, XLEN + off:XLEN + off + 64],
                             ident[0:64, 0:64], start=False, stop=True)
            nc.gpsimd.memset(w_sb[64:128, 0:192], 0.0)
            nc.scalar.copy(out=w_sb[0:64, 0:192], in_=ps[0:64, 0:192])
        win_cache[(r, dj)] = (w_sb, 0)

    def build_windows(keys):
        keys = [k for k in keys
                if k not in win_cache and -2 < k[0] < 64 and k[0] not in (-1, 63)]
        for k in [k for k in set(keys) if False]:
            pass
        seen = []
        for k in keys:
            if k not in seen:
                seen.append(k)
        for a in range(0, len(seen), 2):
            pair = seen[a:a + 2]
            ps = wps.tile([128, 512], fp32, name="wps", tag="w")
            sb = winp.tile([128, 384], bf16, tag="win")
            for s, (r, dj) in enumerate(pair):
                b0 = s * 192
                off = 2 + 64 * r + dj
                nc.tensor.matmul(ps[:, b0:b0 + 128], xb[:, off:off + 128], ident[:],
                                 start=True, stop=False, skip_group_check=(s > 0))
                nc.tensor.matmul(ps[:, b0 + 128:b0 + 192],
                                 xb[0:64, XLEN + off:XLEN + off + 128],
                                 ident[0:64, 0:64], start=False, stop=True,
                                 skip_group_check=(s > 0))
                win_cache[(r, dj)] = (sb, b0)
            nb = len(pair) * 192
            nc.scalar.copy(out=sb[:, 0:nb], in_=ps[:, 0:nb])

    def get_window(r, dj):
        if r <= -2 or r >= 64:
            return (zwin, 0)
        key = (r, dj)
        if key not in win_cache:
            if r in (-1, 63):
                _build_edge(r, dj)
            else:
                build_windows([key])
        return win_cache[key]

    # ---- window & band prefetch ----
    def valid_taps(ti):
        h0 = ti * 2
        return {t: (h0 + t[0] - 2) for t in PE_TAPS if -2 < h0 + t[0] - 2 < 64 or t[0] == 2}

    def wins_for(ti):
        h0 = ti * 2
        for kk in list(win_cache.keys()):
            if kk[0] < h0 - 4:
                del win_cache[kk]
        keys = []
        for i in range(K):
            r = h0 + i - 2
            if r in (-1, 63):
                for j in range(K):
                    if (r, j - 2) not in win_cache:
                        _build_edge(r, j - 2)
            elif -2 < r < 64:
                keys.extend((r, j - 2) for j in range(K))
        build_windows(keys)

    band_store = {}

    def bands(ti):
        for (i, j) in PE_TAPS:
            r = ti * 2 + i - 2
            if r <= -2 or r >= 64:
                continue
            ij = i * K + j
            band = bandp.tile([128, 512], bf16, tag="band")
            # col index f = di*256 + q*2 + t with q = dh*64 + w = band row; the
            # (dh, w) pair merges into one uniform-stride dim (3D ISA limit).
            ov = band[:].rearrange("p (di q t) -> p di q t", di=2, q=128, t=2)
            iv = maskTb[:, ti, ij * 4:ij * 4 + 4].rearrange("p (di t) -> p di t", di=2)
            iv = iv.unsqueeze(2).broadcast_to([128, 2, 128, 2])
            if PE_TAPS[(i, j)] == 'pool':
                for di in range(2):
                    ovd = band[:, di * 256:di * 256 + 256].rearrange(
                        "p (q t) -> p q t", q=128)
                    ivd = maskTb[:, ti, ij * 4 + di * 2:ij * 4 + di * 2 + 2]
                    ivd = ivd.unsqueeze(1).broadcast_to([128, 128, 2])
                    nc.gpsimd.affine_select(out=ovd, in_=ivd, compare_op=AL.is_equal,
                                            fill=0.0, base=0, channel_multiplier=1,
                                            pattern=[[-1, 128], [0, 2]])
            else:
                sv = sel[:].rearrange("p (di q t) -> p di q t", di=2, q=128, t=2)
                nc.vector.tensor_tensor(ov, iv, sv, AL.mult)
            band_store[(ti % 2, (i, j))] = band

    # ---- per-tile reassembly ----
    def taps(ti):
        h0 = ti * 2
        po0 = ops0.tile([128, 512], fp32, name="po0", tag="o0")
        po1 = ops1.tile([64, 512], fp32, name="po1", tag="o1")
        wins = {}
        for i in range(K):
            r = h0 + i - 2
            for j in range(K):
                wins[(i, j)] = get_window(r, j - 2)

        pe_list = [t for t in PE_TAPS if not (wins[t][0] is zwin)]
        # order: start with (2,1), end with (2,2) (always valid, full width)
        pe_list.sort(key=lambda t: (t != (2, 1), t == (2, 2)))
        assert pe_list[-1] == (2, 2) and pe_list[0] == (2, 1)
        dve_list = [t for t in DVE_TAPS if not (wins[t][0] is zwin)]

        # PE taps: first (start, full width)
        for k, (i, j) in enumerate(pe_list[:-1]):
            band = band_store.pop((ti % 2, (i, j)))
            w_sb, wo = wins[(i, j)]
            st = (k == 0)
            nc.tensor.matmul(po0[:], w_sb[:, wo:wo + 128], band[:], start=st, stop=False,
                             skip_group_check=not st)
            nc.tensor.matmul(po1[:], w_sb[:, wo + 128:wo + 192], band[:], start=st, stop=False,
                             skip_group_check=not st)

        # direct taps: DVE STT chains per class, transposed into psum via perms
        for cl in range(4):
            acc = accp.tile([128, 192], bf16, tag="acc")
            fd = True
            for (i, j) in dve_list:
                ij = i * K + j
                col = maskT[:, ti, ij * 4 + cl:ij * 4 + cl + 1]
                src_ = zwin if fd else acc
                w_sb, wo = wins[(i, j)]
                nc.vector.scalar_tensor_tensor(acc[:], w_sb[:, wo:wo + 192], col,
                                               src_[:], AL.mult, AL.add)
                fd = False
            if fd:
                continue
            di, dj = cl // 2, cl % 2
            nc.tensor.matmul(po0[:, di * 256:di * 256 + 256], acc[:, 0:128], perm[:, dj, :],
                             start=False, stop=False, skip_group_check=True)
            nc.tensor.matmul(po1[:, di * 256:di * 256 + 256], acc[:, 128:192], perm[:, dj, :],
                             start=False, stop=False, skip_group_check=True)

        # final PE tap (2,2): stop, full width
        i, j = pe_list[-1]
        band = band_store.pop((ti % 2, (i, j)))
        w_sb, wo = wins[(i, j)]
        nc.tensor.matmul(po0[:], w_sb[:, wo:wo + 128], band[:], start=False, stop=True)
        nc.tensor.matmul(po1[:], w_sb[:, wo + 128:wo + 192], band[:], start=False, stop=True)

        # evict psum -> SBUF staging (DMA cannot read PSUM), then DMA out.
        os0 = ostgp.tile([128, 512], fp32, tag="os0")
        os1 = ostgp.tile([64, 512], fp32, tag="os1")
        nc.scalar.copy(out=os0[:], in_=po0[:])
        nc.scalar.copy(out=os1[:], in_=po1[:])
        for di in range(2):
            ov0 = out_d[0:128, 2 * h0 + di:2 * h0 + di + 3:2, :]
            nc.sync.dma_start(out=ov0, in_=os0[:, di * 256:di * 256 + 256]
                              .rearrange("c (dh q) -> c dh q", dh=2))
            ov1 = out_d[128:192, 2 * h0 + di:2 * h0 + di + 3:2, :]
            nc.sync.dma_start(out=ov1, in_=os1[:, di * 256:di * 256 + 256]
                              .rearrange("c (dh q) -> c dh q", dh=2))

    conv1(0)
    conv2(0)
    conv2(1)
    bands(0)
    wins_for(0)
    for ntk in range(1, 8):
        conv1(ntk)
    for ti in range(NT):
        if ti + 2 < NT:
            conv2(ti + 2)
        if ti + 1 < NT:
            bands(ti + 1)
            wins_for(ti + 1)
        taps(ti)
    es.pop_all().close()


def _host_prep(inputs):
    def fold(w, g, b, m, v):
        s = g / np.sqrt(v + EPS)
        return (w * s[:, None, None, None]).astype(np.float32), (b - m * s).astype(np.float32)

    comp_w_eff, comp_b_eff = fold(inputs["comp_w"], inputs["comp_g"], inputs["comp_b"],
                                  inputs["comp_m"], inputs["comp_v"])
    enc_w_eff, enc_b_eff = fold(inputs["enc_w"], inputs["enc_g"], inputs["enc_b"],
                                inputs["enc_m"], inputs["enc_v"])
    cw = np.ascontiguousarray(comp_w_eff[:, :, 0, 0].T)          # [192, 64]
    cb = comp_b_eff.reshape(Cm, 1)
    ew = np.concatenate([enc_w_eff[:, :, dy, dx].T
                         for dy in range(3) for dx in range(3)], axis=1)  # [64, 900]
    ew = np.ascontiguousarray(ew)
    eb = enc_b_eff.reshape(1, E)
    wv = np.arange(128) % 64
    edge = np.zeros((128, K * K), np.float32)
    for j in range(K):
        ok = (wv + j - 2 >= 0) & (wv + j - 2 < W)
        for i in range(K):
            edge[:, i * K + j] = ok
    idm = np.eye(128, dtype=np.float32)
    sel = np.zeros((128, 512), np.float32)
    for f in range(512):
        dh, w = (f // 128) % 2, (f % 128) // 2
        sel[64 * dh + w, f] = 1.0
    perm = np.zeros((128, 2, 256), np.float32)
    for p in range(128):
        dh, w = p // 64, p % 64
        for dj in range(2):
            perm[p, dj, dh * 128 + 2 * w + dj] = 1.0
    perm = perm.reshape(128, 512)
    return dict(cw=cw, cb=cb, ew=ew, eb=eb, edge=edge, idm=idm, sel=sel, perm=perm)


def kernel(**inputs):
    from concourse.bass_utils import run_bass_kernel_spmd

    inputs = {k: np.asarray(v, dtype=np.float32) for k, v in inputs.items()}
    w = _host_prep(inputs)
    if "nc" not in _prog_cache:
        _prog_cache["nc"] = _build_program()
    nc = _prog_cache["nc"]
    x = inputs["x"]
    in_maps = [dict(x=np.ascontiguousarray(x[b].reshape(C, H * W)), **w) for b in range(B)]
    res = run_bass_kernel_spmd(nc, in_maps, list(range(B)))
    out = np.stack([res.results[b]["out"] for b in range(B)])
    return out
